# revision 16
# baseline (speedup 1.0000x reference)
"""Trainium2 Bass kernel for nn_BCE_for_non_zero.

Reference computation (B=2e6 rows, C=14 labels, 4 label-groups):
    bce  = max(x,0) - x*t + log1p(exp(-|x|))          # = softplus(x) - x*t
    s_t  = per-row sums of t within each label group
    mask = 1 for group-0 labels, else (s_t[group] > 0)
    out  = mean(bce * mask)

Math used here (per row, after sharding):
    sum_c softplus(x_c) = -sum_g ln( prod_{c in g} sigmoid(-x_c) )
because softplus(x) = -ln(sigmoid(-x)) and the per-group products turn
13/14 of the Ln work into cheap f32 multiplies.  With the host permuting
columns so each group is a contiguous block, each per-group product is
ONE contiguous tensor_reduce(op=mult).  The masked total per row is then
    total = -sum_g lnS_g - sum_c x*t + sum_{g!=0} drop_g * lnS_g
with drop_g = (s_t_g == 0) (a dropped group has all t=0 so its bce block
sums to -lnS_g exactly).

Per-core mapping (pure data parallel over rows, 8 cores):
  - rows tiled as [128 partitions, K rows/partition, 14]; per-partition
    contiguous f32 DMA (HWDGE)
  - DVE: fused multiply-reduce (scalar_tensor_tensor, junk output to
    PSUM) for -sum(x*t), in 3 chunks so ACT can start early;
    per-group reduce_mult; drop mask via is_equal; fused multiply-reduce
    for the dropped-group correction
  - ACT: sigmoid(-x) full pass (in place over x), one tiny Ln with fused
    row-sum accumulator
  - GPSIMD: per-group target sums (parallel with DVE/ACT)
Partial sums leave the chip as one [128, n_tiles] f32 tensor per core;
the host permutes columns group-contiguously and reduces outputs in f64.
"""

import numpy as np

C = 14
P = 128
NUM_GROUPS = 4
N_CORES = 8
MAX_K = 651  # rows/partition per tile; 3 tiles cover the 1953 blocks/core
B_CHUNKS = 3  # sub-chunks for the -x*t pass (PSUM junk + early ACT start)

_prog_cache = {}


def _plan_tiles(rows, max_k=MAX_K):
    nb, tail = divmod(rows, P)
    tiles = []
    row0 = 0
    if nb > 0:
        n_full = -(-nb // max_k)
        base, rem = divmod(nb, n_full)
        for i in range(n_full):
            k = base + (1 if i < rem else 0)
            tiles.append((row0, P, k))
            row0 += P * k
    if tail:
        tiles.append((row0, tail, 1))
    return tiles


def _blocks(groups_sorted):
    """(group_id, col_offset, n_cols) for each non-empty group, in order."""
    blocks = []
    for g in range(NUM_GROUPS):
        cols = [c for c in range(C) if groups_sorted[c] == g]
        if cols:
            blocks.append((g, cols[0], len(cols)))
    return blocks


def _chunks(k, n):
    base, rem = divmod(k, n)
    out = []
    o = 0
    for i in range(min(n, k)):
        step = base + (1 if i < rem else 0)
        if step:
            out.append((o, step))
            o += step
    return out


def build_program(rows, groups_sorted):
    import concourse.bacc as bacc
    import concourse.mybir as mybir
    from concourse.tile import TileContext

    f32 = mybir.dt.float32
    mult = mybir.AluOpType.mult
    add = mybir.AluOpType.add
    sub = mybir.AluOpType.subtract
    is_equal = mybir.AluOpType.is_equal
    X = mybir.AxisListType.X

    blocks = _blocks(groups_sorted)
    nblk = len(blocks)
    nz = [b for b in blocks if b[0] != 0]  # non-group-0 blocks
    Gnz = len(nz)
    # offset of the first non-group-0 block in the products tile
    nz_blk0 = next((i for i, b in enumerate(blocks) if b[0] != 0), nblk)

    tiles = _plan_tiles(rows)
    n_tiles = len(tiles)

    nc = bacc.Bacc("TRN2", target_bir_lowering=False, debug=False)
    x_d = nc.dram_tensor("x", [rows, C], f32, kind="ExternalInput")
    t_d = nc.dram_tensor("t", [rows, C], f32, kind="ExternalInput")
    out_d = nc.dram_tensor("out", [P, n_tiles], f32, kind="ExternalOutput")

    with TileContext(nc) as tc:
        with (
            tc.tile_pool(name="big", bufs=2) as big,
            tc.tile_pool(name="prodp", bufs=2) as prodp,
            tc.tile_pool(name="stp", bufs=1) as stp,
            tc.tile_pool(name="smallp", bufs=2) as smallp,
            tc.tile_pool(name="psump", bufs=1, space="PSUM") as psump,
            tc.tile_pool(name="accp", bufs=1) as accp,
        ):
            acc = accp.tile([P, n_tiles], f32, tag="acc")
            nc.vector.memset(acc[:, :], 0.0)

            for j, (row0, p, k) in enumerate(tiles):
                kc = k * C
                xt = big.tile([P, kc], f32, tag="x")
                tt = big.tile([P, kc], f32, tag="t")
                xv = x_d.ap()[row0 : row0 + p * k, :].rearrange(
                    "(p k) c -> p (k c)", p=p
                )
                tv = t_d.ap()[row0 : row0 + p * k, :].rearrange(
                    "(p k) c -> p (k c)", p=p
                )
                nc.sync.dma_start(out=xt[:p, :], in_=xv)
                nc.sync.dma_start(out=tt[:p, :], in_=tv)

                x3 = xt[:p, :].rearrange("p (k c) -> p k c", c=C)
                t3 = tt[:p, :].rearrange("p (k c) -> p k c", c=C)

                sigs = smallp.tile([P, B_CHUNKS + 3], f32, tag="sigs")

                if Gnz:
                    st = stp.tile([P, Gnz * k], f32, tag="st")
                    st3 = st[:p, :].rearrange("p (g k) -> p g k", g=Gnz)
                    # (a) per-group target sums on gpsimd
                    for gi, (g, off, n) in enumerate(nz):
                        dst = st3[:, gi, :]
                        if n == 1:
                            nc.gpsimd.tensor_copy(dst, t3[:, :, off])
                        else:
                            nc.gpsimd.tensor_add(
                                out=dst, in0=t3[:, :, off], in1=t3[:, :, off + 1]
                            )
                            for cx in range(off + 2, off + n):
                                nc.gpsimd.tensor_add(out=dst, in0=dst, in1=t3[:, :, cx])

                # (b)+(c): chunked over k so ACT starts after the first chunk
                chunks = _chunks(k, B_CHUNKS)
                jk = psump.tile(
                    [P, chunks[0][1] * C], f32, tag="junk", space="PSUM"
                )
                for ci, (ko, kn) in enumerate(chunks):
                    sl = slice(ko * C, (ko + kn) * C)
                    # (b) junk <- (x * -1) * t, sigs[ci] = row sums
                    nc.vector.scalar_tensor_tensor(
                        out=jk[:p, : kn * C],
                        in0=xt[:p, sl],
                        scalar=-1.0,
                        in1=tt[:p, sl],
                        op0=mult,
                        op1=mult,
                        accum_out=sigs[:p, ci : ci + 1],
                    )
                    # (c) x <- sigmoid(-x) in place
                    nc.scalar.activation(
                        out=xt[:p, sl],
                        in_=xt[:p, sl],
                        func=mybir.ActivationFunctionType.Sigmoid,
                        scale=-1.0,
                    )

                # (d) per-group products of sigmoid(-x)
                pr = prodp.tile([P, nblk * k], f32, tag="pr")
                for bi, (g, off, n) in enumerate(blocks):
                    nc.vector.tensor_reduce(
                        out=pr[:p, bi * k : (bi + 1) * k],
                        in_=x3[:, :, off : off + n],
                        axis=X,
                        op=mult,
                    )

                # (e) pr <- ln(pr), sigB = sum over all blocks of lnS
                iB = B_CHUNKS
                nc.scalar.activation(
                    out=pr[:p, :],
                    in_=pr[:p, :],
                    func=mybir.ActivationFunctionType.Ln,
                    accum_out=sigs[:p, iB : iB + 1],
                )

                if Gnz:
                    # (f) st <- (st == 0) drop mask
                    nc.vector.tensor_scalar(
                        out=st[:p, :],
                        in0=st[:p, :],
                        scalar1=0.0,
                        scalar2=None,
                        op0=is_equal,
                    )
                    # (g) junk2 <- (drop * 1) * lnS_nz, sigC = row sums
                    # shares the "junk" slot: PSUM only has 8 banks
                    jk2 = psump.tile([P, Gnz * k], f32, tag="junk", space="PSUM")
                    nc.vector.scalar_tensor_tensor(
                        out=jk2[:p, :],
                        in0=st[:p, :],
                        scalar=1.0,
                        in1=pr[:p, nz_blk0 * k : (nz_blk0 + Gnz) * k],
                        op0=mult,
                        op1=mult,
                        accum_out=sigs[:p, iB + 1 : iB + 2],
                    )

                # (h) total = sigA_sum - sigB (+ sigC)
                d1 = sigs[:p, iB + 2 : iB + 3]
                nc.vector.tensor_sub(
                    out=d1, in0=sigs[:p, 0:1], in1=sigs[:p, iB : iB + 1]
                )
                for ci in range(1, len(chunks)):
                    nc.vector.tensor_add(
                        out=d1, in0=d1, in1=sigs[:p, ci : ci + 1]
                    )
                if Gnz:
                    nc.vector.tensor_add(
                        out=acc[:p, j : j + 1],
                        in0=d1,
                        in1=sigs[:p, iB + 1 : iB + 2],
                    )
                else:
                    nc.vector.tensor_copy(acc[:p, j : j + 1], d1)

            nc.sync.dma_start(out=out_d.ap(), in_=acc[:, :])

    nc.compile()
    return nc


def run(inputs, targets, groups, trace=False):
    """Returns (loss, exec_time_ns or None)."""
    from concourse import bass_utils

    B = inputs.shape[0]
    assert inputs.shape[1] == C and B % N_CORES == 0
    rows = B // N_CORES

    groups = np.asarray(groups)
    perm = np.argsort(groups, kind="stable")
    gsort = tuple(int(v) for v in groups[perm])

    key = (rows, gsort)
    if key not in _prog_cache:
        _prog_cache[key] = build_program(rows, gsort)
    nc = _prog_cache[key]

    x = np.ascontiguousarray(np.asarray(inputs, dtype=np.float32)[:, perm])
    t = np.ascontiguousarray(np.asarray(targets, dtype=np.float32)[:, perm])
    in_maps = [
        {
            "x": x[c * rows : (c + 1) * rows],
            "t": t[c * rows : (c + 1) * rows],
        }
        for c in range(N_CORES)
    ]
    res = bass_utils.run_bass_kernel_spmd(
        nc, in_maps, core_ids=list(range(N_CORES)), trace=trace
    )
    total = sum(float(r["out"].astype(np.float64).sum()) for r in res.results)
    return np.float32(total / (B * C)), res.exec_time_ns


def kernel(inputs, targets, groups):
    return run(inputs, targets, groups)[0]


# revision 19
# speedup vs baseline: 1.0750x; 1.0750x over previous
"""Trainium2 Bass kernel for nn_BCE_for_non_zero.

Reference computation (B=2e6 rows, C=14 labels, 4 label-groups):
    bce  = max(x,0) - x*t + log1p(exp(-|x|))          # = softplus(x) - x*t
    s_t  = per-row sums of t within each label group
    mask = 1 for group-0 labels, else (s_t[group] > 0)
    out  = mean(bce * mask)

Math used here (per row, after sharding):
    sum_c softplus(x_c) = -sum_g ln( prod_{c in g} sigmoid(-x_c) )
because softplus(x) = -ln(sigmoid(-x)) and the per-group products turn
13/14 of the Ln work into cheap f32 multiplies.  With the host permuting
columns so each group is a contiguous block, each per-group product is
ONE contiguous tensor_reduce(op=mult).  The masked total per row is then
    total = -sum_g lnS_g - sum_c x*t + sum_{g!=0} drop_g * lnS_g
with drop_g = (s_t_g == 0) (a dropped group has all t=0 so its bce block
sums to -lnS_g exactly).

Per-core mapping (pure data parallel over rows, 8 cores):
  - rows tiled as [128 partitions, K rows/partition, 14]; per-partition
    contiguous f32 DMA (HWDGE)
  - DVE: fused multiply-reduce (scalar_tensor_tensor, junk output to
    PSUM) for -sum(x*t), in 3 chunks so ACT can start early;
    per-group reduce_mult; drop mask via is_equal; fused multiply-reduce
    for the dropped-group correction
  - ACT: sigmoid(-x) full pass (in place over x), one tiny Ln with fused
    row-sum accumulator
  - GPSIMD: per-group target sums (parallel with DVE/ACT)
Partial sums leave the chip as one [128, n_tiles] f32 tensor per core;
the host permutes columns group-contiguously and reduces outputs in f64.
"""

import numpy as np

C = 14
P = 128
NUM_GROUPS = 4
N_CORES = 8
MAX_K = 489  # rows/partition per tile; 4 tiles cover the 1953 blocks/core
B_CHUNKS = 2  # sub-chunks for the -x*t pass (PSUM junk + early ACT start)

_prog_cache = {}


def _plan_tiles(rows, max_k=MAX_K):
    nb, tail = divmod(rows, P)
    tiles = []
    row0 = 0
    if nb > 0:
        n_full = -(-nb // max_k)
        base, rem = divmod(nb, n_full)
        for i in range(n_full):
            k = base + (1 if i < rem else 0)
            tiles.append((row0, P, k))
            row0 += P * k
    if tail:
        tiles.append((row0, tail, 1))
    return tiles


def _blocks(groups_sorted):
    """(group_id, col_offset, n_cols) for each non-empty group, in order."""
    blocks = []
    for g in range(NUM_GROUPS):
        cols = [c for c in range(C) if groups_sorted[c] == g]
        if cols:
            blocks.append((g, cols[0], len(cols)))
    return blocks


def _chunks(k, n):
    base, rem = divmod(k, n)
    out = []
    o = 0
    for i in range(min(n, k)):
        step = base + (1 if i < rem else 0)
        if step:
            out.append((o, step))
            o += step
    return out


def build_program(rows, groups_sorted):
    import concourse.bacc as bacc
    import concourse.mybir as mybir
    from concourse.tile import TileContext

    f32 = mybir.dt.float32
    mult = mybir.AluOpType.mult
    add = mybir.AluOpType.add
    sub = mybir.AluOpType.subtract
    is_equal = mybir.AluOpType.is_equal
    X = mybir.AxisListType.X

    blocks = _blocks(groups_sorted)
    nblk = len(blocks)
    nz = [b for b in blocks if b[0] != 0]  # non-group-0 blocks
    Gnz = len(nz)
    # offset of the first non-group-0 block in the products tile
    nz_blk0 = next((i for i, b in enumerate(blocks) if b[0] != 0), nblk)

    tiles = _plan_tiles(rows)
    n_tiles = len(tiles)

    nc = bacc.Bacc("TRN2", target_bir_lowering=False, debug=False)
    x_d = nc.dram_tensor("x", [rows, C], f32, kind="ExternalInput")
    t_d = nc.dram_tensor("t", [rows, C], f32, kind="ExternalInput")
    out_d = nc.dram_tensor("out", [P, n_tiles], f32, kind="ExternalOutput")

    with TileContext(nc) as tc:
        with (
            tc.tile_pool(name="big", bufs=2) as big,
            tc.tile_pool(name="prodp", bufs=2) as prodp,
            tc.tile_pool(name="stp", bufs=1) as stp,
            tc.tile_pool(name="smallp", bufs=2) as smallp,
            tc.tile_pool(name="psump", bufs=1, space="PSUM") as psump,
            tc.tile_pool(name="accp", bufs=1) as accp,
        ):
            acc = accp.tile([P, n_tiles], f32, tag="acc")
            nc.vector.memset(acc[:, :], 0.0)

            for j, (row0, p, k) in enumerate(tiles):
                kc = k * C
                xt = big.tile([P, kc], f32, tag="x")
                tt = big.tile([P, kc], f32, tag="t")
                xv = x_d.ap()[row0 : row0 + p * k, :].rearrange(
                    "(p k) c -> p (k c)", p=p
                )
                tv = t_d.ap()[row0 : row0 + p * k, :].rearrange(
                    "(p k) c -> p (k c)", p=p
                )
                # t first: it feeds the slowest stage (gpsimd group sums)
                nc.sync.dma_start(out=tt[:p, :], in_=tv)
                nc.sync.dma_start(out=xt[:p, :], in_=xv)

                x3 = xt[:p, :].rearrange("p (k c) -> p k c", c=C)
                t3 = tt[:p, :].rearrange("p (k c) -> p k c", c=C)

                sigs = smallp.tile([P, B_CHUNKS + 3], f32, tag="sigs")

                # small tiles pay gpsimd's ~1.3us/op dispatch; do them on DVE
                st_on_dve = p < P or k < 64
                if Gnz:
                    st = stp.tile([P, Gnz * k], f32, tag="st")
                    st3 = st[:p, :].rearrange("p (g k) -> p g k", g=Gnz)
                    if st_on_dve:
                        # (a') contiguous per-group reduce-adds on DVE
                        for gi, (g, off, n) in enumerate(nz):
                            nc.vector.tensor_reduce(
                                out=st3[:, gi, :],
                                in_=t3[:, :, off : off + n],
                                axis=X,
                                op=add,
                            )
                    else:
                        # (a) per-group target sums on gpsimd, pair-merged:
                        # one op sums column-pairs for two halves at once
                        scr = stp.tile([P, 2 * k], f32, tag="scr")
                        s3 = scr[:p, :].rearrange("p (h k) -> p h k", h=2)
                        for gi, (g, off, n) in enumerate(nz):
                            dst = st3[:, gi, :]
                            if n == 1:
                                nc.gpsimd.tensor_copy(dst, t3[:, :, off])
                            elif n == 2:
                                nc.gpsimd.tensor_add(
                                    out=dst, in0=t3[:, :, off], in1=t3[:, :, off + 1]
                                )
                            elif n == 3:
                                nc.gpsimd.tensor_add(
                                    out=dst, in0=t3[:, :, off], in1=t3[:, :, off + 1]
                                )
                                nc.gpsimd.tensor_add(
                                    out=dst, in0=dst, in1=t3[:, :, off + 2]
                                )
                            else:
                                # n in {4, 5}: pairwise [p, 2, k] add, fold, tail
                                nc.gpsimd.tensor_add(
                                    out=s3[:, :, :],
                                    in0=t3[:, :, off : off + 2].rearrange(
                                        "p k h -> p h k"
                                    ),
                                    in1=t3[:, :, off + 2 : off + 4].rearrange(
                                        "p k h -> p h k"
                                    ),
                                )
                                nc.gpsimd.tensor_add(
                                    out=dst, in0=s3[:, 0, :], in1=s3[:, 1, :]
                                )
                                for cx in range(off + 4, off + n):
                                    nc.gpsimd.tensor_add(
                                        out=dst, in0=dst, in1=t3[:, :, cx]
                                    )

                # (b)+(c): chunked over k so ACT starts after the first chunk
                chunks = _chunks(k, B_CHUNKS)
                jk = psump.tile(
                    [P, chunks[0][1] * C], f32, tag="junk", space="PSUM"
                )
                for ci, (ko, kn) in enumerate(chunks):
                    sl = slice(ko * C, (ko + kn) * C)
                    # (b) junk <- (x * -1) * t, sigs[ci] = row sums
                    nc.vector.scalar_tensor_tensor(
                        out=jk[:p, : kn * C],
                        in0=xt[:p, sl],
                        scalar=-1.0,
                        in1=tt[:p, sl],
                        op0=mult,
                        op1=mult,
                        accum_out=sigs[:p, ci : ci + 1],
                    )
                    # (c) x <- sigmoid(-x) in place
                    nc.scalar.activation(
                        out=xt[:p, sl],
                        in_=xt[:p, sl],
                        func=mybir.ActivationFunctionType.Sigmoid,
                        scale=-1.0,
                    )

                # (d) per-group products of sigmoid(-x)
                pr = prodp.tile([P, nblk * k], f32, tag="pr")
                for bi, (g, off, n) in enumerate(blocks):
                    nc.vector.tensor_reduce(
                        out=pr[:p, bi * k : (bi + 1) * k],
                        in_=x3[:, :, off : off + n],
                        axis=X,
                        op=mult,
                    )

                # (e) pr <- ln(pr), sigB = sum over all blocks of lnS
                iB = B_CHUNKS
                nc.scalar.activation(
                    out=pr[:p, :],
                    in_=pr[:p, :],
                    func=mybir.ActivationFunctionType.Ln,
                    accum_out=sigs[:p, iB : iB + 1],
                )

                if Gnz:
                    # (f) st <- (st == 0) drop mask
                    nc.vector.tensor_scalar(
                        out=st[:p, :],
                        in0=st[:p, :],
                        scalar1=0.0,
                        scalar2=None,
                        op0=is_equal,
                    )
                    # (g) junk2 <- (drop * 1) * lnS_nz, sigC = row sums
                    # shares the "junk" slot: PSUM only has 8 banks
                    jk2 = psump.tile([P, Gnz * k], f32, tag="junk", space="PSUM")
                    nc.vector.scalar_tensor_tensor(
                        out=jk2[:p, :],
                        in0=st[:p, :],
                        scalar=1.0,
                        in1=pr[:p, nz_blk0 * k : (nz_blk0 + Gnz) * k],
                        op0=mult,
                        op1=mult,
                        accum_out=sigs[:p, iB + 1 : iB + 2],
                    )

                # (h) total = sigA_sum - sigB (+ sigC)
                d1 = sigs[:p, iB + 2 : iB + 3]
                nc.vector.tensor_sub(
                    out=d1, in0=sigs[:p, 0:1], in1=sigs[:p, iB : iB + 1]
                )
                for ci in range(1, len(chunks)):
                    nc.vector.tensor_add(
                        out=d1, in0=d1, in1=sigs[:p, ci : ci + 1]
                    )
                if Gnz:
                    nc.vector.tensor_add(
                        out=acc[:p, j : j + 1],
                        in0=d1,
                        in1=sigs[:p, iB + 1 : iB + 2],
                    )
                else:
                    nc.vector.tensor_copy(acc[:p, j : j + 1], d1)

            nc.sync.dma_start(out=out_d.ap(), in_=acc[:, :])

    nc.compile()
    return nc


def run(inputs, targets, groups, trace=False):
    """Returns (loss, exec_time_ns or None)."""
    from concourse import bass_utils

    B = inputs.shape[0]
    assert inputs.shape[1] == C and B % N_CORES == 0
    rows = B // N_CORES

    groups = np.asarray(groups)
    perm = np.argsort(groups, kind="stable")
    gsort = tuple(int(v) for v in groups[perm])

    key = (rows, gsort)
    if key not in _prog_cache:
        _prog_cache[key] = build_program(rows, gsort)
    nc = _prog_cache[key]

    x = np.ascontiguousarray(np.asarray(inputs, dtype=np.float32)[:, perm])
    t = np.ascontiguousarray(np.asarray(targets, dtype=np.float32)[:, perm])
    in_maps = [
        {
            "x": x[c * rows : (c + 1) * rows],
            "t": t[c * rows : (c + 1) * rows],
        }
        for c in range(N_CORES)
    ]
    res = bass_utils.run_bass_kernel_spmd(
        nc, in_maps, core_ids=list(range(N_CORES)), trace=trace
    )
    total = sum(float(r["out"].astype(np.float64).sum()) for r in res.results)
    return np.float32(total / (B * C)), res.exec_time_ns


def kernel(inputs, targets, groups):
    return run(inputs, targets, groups)[0]
